# revision 7
# baseline (speedup 1.0000x reference)
"""CacheMHA decode kernel for 8 TRN2 NeuronCores.

Sharding: data-parallel over batch (B=32 -> 4 per core). Each core computes the
full attention step for its 4 sequences (all 16 heads) plus the cache
concatenation, so no collectives are needed.

Layout strategy (per core):
  - cache K/V for a (batch, head-pair) is streamed HBM->SBUF as one 2 MiB DMA
    with 8 KiB contiguous runs (partition p holds seq rows p*32..p*32+31), and
    the same SBUF tile is written back to the output cache (fused copy: each
    cache byte is read once and written once).
  - q.K dots run on the vector engine (tensor_mul + tensor_reduce over the
    head_dim axis); exp on the scalar engine with a fused per-partition sum
    (softmax denominators) accumulated into a [128, 64] tile.
  - cross-partition sums (denominator, attn.V) use single tensor-engine
    matmuls against a ones vector.
  - projections x@W.T run on the vector engine as broadcast-multiply-reduce
    against W row blocks, avoiding any W transposes.
"""

import numpy as np

D = 1024
H = 16
HD = 64
B = 32
KV = 4096
N_CORES = 8
BPC = B // N_CORES  # batches per core

_CACHED_NC = None


def _build():
    global _CACHED_NC
    if _CACHED_NC is not None:
        return _CACHED_NC

    import concourse.bacc as bacc
    import concourse.mybir as mybir
    import concourse.tile as tile
    from concourse.masks import make_identity

    FP = mybir.dt.float32
    AX = mybir.AxisListType
    OP = mybir.AluOpType
    ACT = mybir.ActivationFunctionType

    nc = bacc.Bacc(
        "TRN2", target_bir_lowering=False, debug=False, num_devices=N_CORES
    )
    x_d = nc.declare_dram_parameter("x", [BPC, 1, D], FP, isOutput=False)
    ck_d = nc.declare_dram_parameter("cache_k", [BPC, H, KV, HD], FP, isOutput=False)
    cv_d = nc.declare_dram_parameter("cache_v", [BPC, H, KV, HD], FP, isOutput=False)
    wq_d = nc.declare_dram_parameter("Wq", [D, D], FP, isOutput=False)
    wk_d = nc.declare_dram_parameter("Wk", [D, D], FP, isOutput=False)
    wv_d = nc.declare_dram_parameter("Wv", [D, D], FP, isOutput=False)
    wo_d = nc.declare_dram_parameter("Wo", [D, D], FP, isOutput=False)
    out_d = nc.declare_dram_parameter("out", [BPC, 1, D], FP, isOutput=True)
    ko_d = nc.declare_dram_parameter("k", [BPC, H, KV + 1, HD], FP, isOutput=True)
    vo_d = nc.declare_dram_parameter("v", [BPC, H, KV + 1, HD], FP, isOutput=True)

    with tile.TileContext(nc) as tc:
        with (
            tc.tile_pool(name="consts", bufs=1) as consts,
            tc.tile_pool(name="small", bufs=2) as small,
            tc.tile_pool(name="kvpool", bufs=2) as kvp,
            tc.tile_pool(name="tmppool", bufs=2) as tmpp,
            tc.tile_pool(name="wpool", bufs=2) as wpool,
            tc.tile_pool(name="ps_qb", bufs=2, space="PSUM") as ps_qb,
            tc.tile_pool(name="ps_y", bufs=2, space="PSUM") as ps_y,
            tc.tile_pool(name="ps_misc", bufs=2, space="PSUM") as ps_misc,
        ):
            ones_r = consts.tile([1, 128], FP, tag="ones_r")  # row of ones
            ones_c = consts.tile([128, 1], FP, tag="ones_c")  # column of ones
            ident = consts.tile([128, 128], FP, tag="ident")
            nc.gpsimd.memset(ones_r[:], 1.0)
            nc.gpsimd.memset(ones_c[:], 1.0)
            make_identity(nc, ident[:])

            # ---- load x, build x_row0 [1, 4096] and x broadcast [128, 4096]
            x_sb = consts.tile([BPC, D], FP, tag="x_sb")
            nc.sync.dma_start(x_sb[:], x_d[:, 0, :])
            x_row0 = consts.tile([1, BPC * D], FP, tag="x_row0")
            nc.sync.dma_start(x_row0[0:1, :], x_sb[:])
            x_bc = consts.tile([128, BPC * D], FP, tag="x_bc")
            for bi in range(BPC):
                for hf in range(2):
                    off = bi * D + hf * 512
                    pb = ps_misc.tile([128, 512], FP, tag="misc")
                    nc.tensor.matmul(
                        pb[:], lhsT=ones_r[:], rhs=x_row0[0:1, off : off + 512],
                        start=True, stop=True,
                    )
                    nc.scalar.copy(x_bc[:, off : off + 512], pb[:])

            # ---- projections q/k/v: projT[:, m*4+b] = (W[m*128:(m+1)*128] * x_b).sum
            projTs = {}
            for wname, w_d in (("q", wq_d), ("k", wk_d), ("v", wv_d)):
                projT = consts.tile([128, 8 * BPC], FP, tag=f"{wname}T")
                projTs[wname] = projT
                for m in range(8):
                    wblk = wpool.tile([128, D], FP, tag="wblk")
                    nc.sync.dma_start(wblk[:], w_d[m * 128 : (m + 1) * 128, :])
                    for bi in range(BPC):
                        scr = tmpp.tile([128, D], FP, tag="tm")
                        nc.vector.tensor_mul(
                            scr[:], wblk[:], x_bc[:, bi * D : (bi + 1) * D]
                        )
                        nc.vector.tensor_reduce(
                            projT[:, m * BPC + bi : m * BPC + bi + 1],
                            scr[:], axis=AX.X, op=OP.add,
                        )

            # ---- transpose projT chunks into row layout [BPC, D], then row0 [1, 4096]
            row0 = {}
            for wname in ("q", "k", "v"):
                r = consts.tile([BPC, D], FP, tag="rows")
                for m in range(8):
                    tp = ps_misc.tile([BPC, 128], FP, tag="misc")
                    nc.tensor.transpose(
                        tp[:], projTs[wname][:, m * BPC : (m + 1) * BPC], ident[:]
                    )
                    nc.scalar.copy(r[:, m * 128 : (m + 1) * 128], tp[:])
                r0 = consts.tile([1, BPC * D], FP, tag=f"{wname}_row0")
                row0[wname] = r0
                nc.sync.dma_start(r0[0:1, :], r[:])

            # ---- write new k/v rows into the output caches at seq position KV
            nc.scalar.dma_start(ko_d[:, :, KV, :], row0["k"][0:1, :])
            nc.scalar.dma_start(vo_d[:, :, KV, :], row0["v"][0:1, :])

            # ---- new-token scores: pnew[bh] = exp(0.125 * q_bh . knew_bh)
            sn_tmp = consts.tile([1, BPC * D], FP, tag="x_row0")
            nc.vector.tensor_mul(sn_tmp[:], row0["k"][:], row0["q"][:])
            snew = consts.tile([1, BPC * H], FP, tag="snew")
            nc.vector.tensor_reduce(
                snew[:], sn_tmp[:].rearrange("p (a d) -> p a d", d=HD),
                axis=AX.X, op=OP.add,
            )
            pnew = consts.tile([1, BPC * H], FP, tag="pnew")
            nc.scalar.activation(pnew[:], snew[:], ACT.Exp, scale=0.125)

            # ---- main loop over (batch, head-pair): stream cache, attention
            partials = consts.tile([128, BPC * H], FP, tag="partials")
            y_buf = consts.tile([HD, BPC * H], FP, tag="y_buf")
            HT = KV // 128 * HD  # free size per head: 32 * 64 = 2048

            for bi in range(BPC):
                for hp in range(H // 2):
                    h0 = 2 * hp
                    kt = kvp.tile([128, 2 * HT], FP, tag="kt")
                    kt4 = kt[:].rearrange("p (h t d) -> p h t d", h=2, d=HD)
                    nc.sync.dma_start(
                        kt4,
                        ck_d[bi, h0 : h0 + 2].rearrange("h (p t) d -> p h t d", p=128),
                    )
                    nc.scalar.dma_start(
                        ko_d[bi, h0 : h0 + 2, 0:KV, :].rearrange(
                            "h (p t) d -> p h t d", p=128
                        ),
                        kt4,
                    )
                    vt = kvp.tile([128, 2 * HT], FP, tag="vt")
                    nc.sync.dma_start(
                        vt[:],
                        cv_d[bi, h0 : h0 + 2].rearrange("h (p t) d -> p h t d", p=128),
                    )
                    nc.scalar.dma_start(
                        vo_d[bi, h0 : h0 + 2, 0:KV, :].rearrange(
                            "h (p t) d -> p h t d", p=128
                        ),
                        vt[:],
                    )
                    for j in range(2):
                        h = h0 + j
                        bh = bi * H + h
                        ksl = kt[:, j * HT : (j + 1) * HT].rearrange(
                            "p (t d) -> p t d", d=HD
                        )
                        vsl = vt[:, j * HT : (j + 1) * HT].rearrange(
                            "p (t d) -> p t d", d=HD
                        )
                        # broadcast q_bh across partitions via ones matmul
                        qb = ps_qb.tile([128, HD], FP, tag="qb")
                        nc.tensor.matmul(
                            qb[:], lhsT=ones_r[:],
                            rhs=row0["q"][0:1, bh * HD : (bh + 1) * HD],
                            start=True, stop=True,
                        )
                        tm = tmpp.tile([128, HT], FP, tag="tm")
                        t3 = tm[:].rearrange("p (t d) -> p t d", d=HD)
                        nc.vector.tensor_mul(
                            t3, ksl, qb[:, None, :].broadcast_to((128, KV // 128, HD))
                        )
                        scores = small.tile([128, KV // 128], FP, tag="scores")
                        nc.vector.tensor_reduce(scores[:], t3, axis=AX.X, op=OP.add)
                        pt = small.tile([128, KV // 128], FP, tag="pt")
                        nc.scalar.activation(
                            pt[:], scores[:], ACT.Exp, scale=0.125,
                            accum_out=partials[:, bh : bh + 1],
                        )
                        tm2 = tmpp.tile([128, HT], FP, tag="tm")
                        t23 = tm2[:].rearrange("p (t d) -> p t d", d=HD)
                        nc.vector.tensor_mul(
                            t23, vsl, pt[:, :, None].broadcast_to((128, KV // 128, HD))
                        )
                        pv = small.tile([128, HD], FP, tag="pv")
                        nc.vector.tensor_reduce(
                            pv[:], tm2[:].rearrange("p (t d) -> p d t", d=HD),
                            axis=AX.X, op=OP.add,
                        )
                        yp = ps_y.tile([HD, 1], FP, tag="yp")
                        nc.tensor.matmul(
                            yp[:], lhsT=pv[:], rhs=ones_c[:], start=True, stop=False
                        )
                        nc.tensor.matmul(
                            yp[:],
                            lhsT=row0["v"][0:1, bh * HD : (bh + 1) * HD],
                            rhs=pnew[0:1, bh : bh + 1],
                            start=False, stop=True,
                        )
                        nc.scalar.copy(y_buf[:, bh : bh + 1], yp[:])

            # ---- softmax denominators and normalization
            dn = ps_misc.tile([BPC * H, 1], FP, tag="misc")
            nc.tensor.matmul(dn[:], lhsT=partials[:], rhs=ones_c[:], start=True, stop=False)
            nc.tensor.matmul(
                dn[:], lhsT=pnew[:], rhs=ones_r[0:1, 0:1], start=False, stop=True
            )
            dn_sb = small.tile([BPC * H, 1], FP, tag="dn_sb")
            nc.vector.tensor_copy(dn_sb[:], dn[:])
            inv = consts.tile([BPC * H, 1], FP, tag="inv")
            nc.vector.reciprocal(inv[:], dn_sb[:])

            yT = ps_misc.tile([BPC * H, HD], FP, tag="misc")
            nc.tensor.transpose(yT[:], y_buf[:], ident[0:HD, 0:HD])
            y_byh = consts.tile([BPC * H, HD], FP, tag="y_byh")
            # out = in * inv[partition] : per-(b,h) softmax normalization
            nc.scalar.activation(y_byh[:], yT[:], ACT.Copy, scale=inv[:])
            y_row0 = consts.tile([1, BPC * D], FP, tag="x_row0")
            nc.sync.dma_start(y_row0[0:1, :], y_byh[:])

            # ---- output projection out = y @ Wo.T
            y_bc = consts.tile([128, BPC * D], FP, tag="x_bc")
            for bi in range(BPC):
                for hf in range(2):
                    off = bi * D + hf * 512
                    pb = ps_misc.tile([128, 512], FP, tag="misc")
                    nc.tensor.matmul(
                        pb[:], lhsT=ones_r[:], rhs=y_row0[0:1, off : off + 512],
                        start=True, stop=True,
                    )
                    nc.scalar.copy(y_bc[:, off : off + 512], pb[:])
            outT = consts.tile([128, 8 * BPC], FP, tag="qT")
            for m in range(8):
                wblk = wpool.tile([128, D], FP, tag="wblk")
                nc.sync.dma_start(wblk[:], wo_d[m * 128 : (m + 1) * 128, :])
                for bi in range(BPC):
                    scr = tmpp.tile([128, D], FP, tag="tm")
                    nc.vector.tensor_mul(
                        scr[:], wblk[:], y_bc[:, bi * D : (bi + 1) * D]
                    )
                    nc.vector.tensor_reduce(
                        outT[:, m * BPC + bi : m * BPC + bi + 1],
                        scr[:], axis=AX.X, op=OP.add,
                    )
            out_buf = consts.tile([BPC, D], FP, tag="rows")
            for m in range(8):
                tp = ps_misc.tile([BPC, 128], FP, tag="misc")
                nc.tensor.transpose(tp[:], outT[:, m * BPC : (m + 1) * BPC], ident[:])
                nc.scalar.copy(out_buf[:, m * 128 : (m + 1) * 128], tp[:])
            nc.sync.dma_start(out_d[:, 0, :], out_buf[:])

    nc.compile()
    _CACHED_NC = nc
    return nc


def _in_maps(x, cache_k, cache_v, Wq, Wk, Wv, Wo):
    x = np.asarray(x, dtype=np.float32)
    cache_k = np.asarray(cache_k, dtype=np.float32)
    cache_v = np.asarray(cache_v, dtype=np.float32)
    Wq = np.ascontiguousarray(np.asarray(Wq, dtype=np.float32))
    Wk = np.ascontiguousarray(np.asarray(Wk, dtype=np.float32))
    Wv = np.ascontiguousarray(np.asarray(Wv, dtype=np.float32))
    Wo = np.ascontiguousarray(np.asarray(Wo, dtype=np.float32))

    in_maps = []
    for c in range(N_CORES):
        sl = slice(c * BPC, (c + 1) * BPC)
        in_maps.append(
            {
                "x": np.ascontiguousarray(x[sl]),
                "cache_k": np.ascontiguousarray(cache_k[sl]),
                "cache_v": np.ascontiguousarray(cache_v[sl]),
                "Wq": Wq,
                "Wk": Wk,
                "Wv": Wv,
                "Wo": Wo,
            }
        )
    return in_maps


def _gather(results):
    out = np.concatenate([r["out"] for r in results], axis=0)
    k = np.concatenate([r["k"] for r in results], axis=0)
    v = np.concatenate([r["v"] for r in results], axis=0)
    return out, k, v


def kernel(x, cache_k, cache_v, Wq, Wk, Wv, Wo):
    from concourse.bass_utils import run_bass_kernel_spmd

    nc = _build()
    in_maps = _in_maps(x, cache_k, cache_v, Wq, Wk, Wv, Wo)
    res = run_bass_kernel_spmd(nc, in_maps, core_ids=list(range(N_CORES))).results
    return _gather(res)


# revision 10
# speedup vs baseline: 1.9763x; 1.9763x over previous
"""CacheMHA decode kernel for 8 TRN2 NeuronCores.

Sharding: data-parallel over batch (B=32 -> 4 per core). Each core computes the
full attention step for its 4 sequences (all 16 heads) plus the cache
concatenation, so no collectives are needed.

Per-core design (engine balance is the point):
  - DMA (the roofline, ~270 MB/core): cache K/V per (batch, head) streams
    HBM->SBUF as one 1 MiB DMA with 8 KiB contiguous runs (partition p holds
    seq rows p*32..p*32+31); the same SBUF tile is written back to the output
    cache, so each cache byte is read once and written once. Loads ride the
    SP HWDGE ring, stores the Activation ring.
  - DVE does only the q.K dot products (multiply + segmented reduce).
  - PE does everything contraction-shaped: weight transposes, projections,
    q broadcast, attn.V (PSUM-accumulated column matmuls), per-head softmax
    denominator sums, and the output projection.
  - ACT does only exp (fused denominator accumulation) and issues the store
    DMAs, so its function table never thrashes.
  - Softmax is normalized per-head as soon as that head's denominator is
    known, which removes any serial tail except the small output projection.
"""

import numpy as np

D = 1024
H = 16
HD = 64
B = 32
KV = 4096
N_CORES = 8
BPC = B // N_CORES  # batches per core

_CACHED_NC = None
_SKIP = frozenset()  # experiment toggles: "scores", "attnv", "tail"


def _build():
    global _CACHED_NC
    if _CACHED_NC is not None:
        return _CACHED_NC

    import concourse.bacc as bacc
    import concourse.mybir as mybir
    import concourse.tile as tile
    from concourse.masks import make_identity

    FP = mybir.dt.float32
    AX = mybir.AxisListType
    OP = mybir.AluOpType
    ACT = mybir.ActivationFunctionType

    nc = bacc.Bacc(
        "TRN2", target_bir_lowering=False, debug=False, num_devices=N_CORES
    )
    x_d = nc.declare_dram_parameter("x", [BPC, 1, D], FP, isOutput=False)
    ck_d = nc.declare_dram_parameter("cache_k", [BPC, H, KV, HD], FP, isOutput=False)
    cv_d = nc.declare_dram_parameter("cache_v", [BPC, H, KV, HD], FP, isOutput=False)
    wq_d = nc.declare_dram_parameter("Wq", [D, D], FP, isOutput=False)
    wk_d = nc.declare_dram_parameter("Wk", [D, D], FP, isOutput=False)
    wv_d = nc.declare_dram_parameter("Wv", [D, D], FP, isOutput=False)
    wo_d = nc.declare_dram_parameter("Wo", [D, D], FP, isOutput=False)
    out_d = nc.declare_dram_parameter("out", [BPC, 1, D], FP, isOutput=True)
    ko_d = nc.declare_dram_parameter("k", [BPC, H, KV + 1, HD], FP, isOutput=True)
    vo_d = nc.declare_dram_parameter("v", [BPC, H, KV + 1, HD], FP, isOutput=True)

    HT = KV // 128 * HD  # free size per head tile: 32*64 = 2048 f32
    NT = KV // 128  # 32 seq sub-rows per partition

    with tile.TileContext(nc) as tc:
        with (
            tc.tile_pool(name="consts", bufs=1) as consts,
            tc.tile_pool(name="small", bufs=3) as small,
            tc.tile_pool(name="kvpool", bufs=3) as kvp,
            tc.tile_pool(name="tmppool", bufs=2) as tmpp,
            tc.tile_pool(name="wpool", bufs=2) as wpool,
            tc.tile_pool(name="ps_qb", bufs=2, space="PSUM") as ps_qb,
            tc.tile_pool(name="ps_y", bufs=2, space="PSUM") as ps_y,
            tc.tile_pool(name="ps_proj", bufs=2, space="PSUM") as ps_proj,
            tc.tile_pool(name="ps_misc", bufs=2, space="PSUM") as ps_misc,
        ):
            ones_r = consts.tile([1, 128], FP, tag="ones_r")  # row of ones
            ones_c = consts.tile([128, 1], FP, tag="ones_c")  # column of ones
            ident = consts.tile([128, 128], FP, tag="ident")
            nc.gpsimd.memset(ones_r[:], 1.0)
            nc.gpsimd.memset(ones_c[:], 1.0)
            make_identity(nc, ident[:])

            # ---- x -> xT chunks [128, 4] x 8 (transpose on PE)
            x_sb = consts.tile([BPC, D], FP, tag="x_sb")
            nc.sync.dma_start(x_sb[:], x_d[:, 0, :])
            xT_all = consts.tile([128, 8 * BPC], FP, tag="xT_all")
            for k_ in range(8):
                tp = ps_misc.tile([128, BPC], FP, tag="misc")
                nc.tensor.transpose(
                    tp[:], x_sb[:, k_ * 128 : (k_ + 1) * 128], ident[0:BPC, 0:BPC]
                )
                nc.vector.tensor_copy(xT_all[:, k_ * BPC : (k_ + 1) * BPC], tp[:])

            # ---- projections q/k/v on PE: transpose W blocks, accumulate
            projTs = {}
            for wname, w_d in (("q", wq_d), ("k", wk_d), ("v", wv_d)):
                projT = consts.tile([128, 8 * BPC], FP, tag=f"{wname}T")
                projTs[wname] = projT
                for m in range(8):
                    wblk = wpool.tile([128, D], FP, tag="wblk")
                    nc.sync.dma_start(wblk[:], w_d[m * 128 : (m + 1) * 128, :])
                    pp = ps_proj.tile([128, BPC], FP, tag="proj")
                    for k_ in range(8):
                        wt_ps = ps_misc.tile([128, 128], FP, tag="misc")
                        nc.tensor.transpose(
                            wt_ps[:], wblk[:, k_ * 128 : (k_ + 1) * 128], ident[:]
                        )
                        wt_sb = small.tile([128, 128], FP, tag="wt_sb")
                        nc.vector.tensor_copy(wt_sb[:], wt_ps[:])
                        nc.tensor.matmul(
                            pp[:],
                            lhsT=wt_sb[:],
                            rhs=xT_all[:, k_ * BPC : (k_ + 1) * BPC],
                            start=(k_ == 0),
                            stop=(k_ == 7),
                        )
                    nc.vector.tensor_copy(projT[:, m * BPC : (m + 1) * BPC], pp[:])

            # ---- projT -> row layout [BPC, D] -> row0 [1, 4096]
            row0 = {}
            for wname in ("q", "k", "v"):
                r = consts.tile([BPC, D], FP, tag="rows")
                for m in range(8):
                    tp = ps_misc.tile([128, 512], FP, tag="misc")
                    nc.tensor.transpose(
                        tp[0:BPC, 0:128],
                        projTs[wname][:, m * BPC : (m + 1) * BPC],
                        ident[:],
                    )
                    nc.vector.tensor_copy(
                        r[:, m * 128 : (m + 1) * 128], tp[0:BPC, 0:128]
                    )
                r0 = consts.tile([1, BPC * D], FP, tag=f"{wname}_row0")
                row0[wname] = r0
                nc.sync.dma_start(r0[0:1, :], r[:])

            # ---- write new k/v rows into the output caches at seq position KV
            nc.scalar.dma_start(ko_d[:, :, KV, :], row0["k"][0:1, :])
            nc.scalar.dma_start(vo_d[:, :, KV, :], row0["v"][0:1, :])

            # ---- new-token scores: pnew[bh] = exp(0.125 * q_bh . knew_bh)
            sn_tmp = consts.tile([1, BPC * D], FP, tag="sn_tmp")
            nc.vector.tensor_mul(sn_tmp[:], row0["k"][:], row0["q"][:])
            snew = consts.tile([1, BPC * H], FP, tag="snew")
            nc.vector.tensor_reduce(
                snew[:], sn_tmp[:].rearrange("p (a d) -> p a d", d=HD),
                axis=AX.X, op=OP.add,
            )
            pnew = consts.tile([1, BPC * H], FP, tag="pnew")
            nc.scalar.activation(pnew[:], snew[:], ACT.Exp, scale=0.125)

            # ---- main loop over (batch, head): stream cache, attention
            partials = consts.tile([128, BPC * H], FP, tag="partials")
            y_buf = consts.tile([HD, BPC * H], FP, tag="y_buf")  # normalized y^T

            for bi in range(BPC):
                for h in range(H):
                    bh = bi * H + h
                    kt = kvp.tile([128, HT], FP, tag="kt")
                    kt3 = kt[:].rearrange("p (t d) -> p t d", d=HD)
                    nc.sync.dma_start(
                        kt3, ck_d[bi, h].rearrange("(p t) d -> p t d", p=128)
                    )
                    nc.scalar.dma_start(
                        ko_d[bi, h, 0:KV, :].rearrange("(p t) d -> p t d", p=128),
                        kt3,
                    )
                    vt = kvp.tile([128, HT], FP, tag="vt")
                    vt3 = vt[:].rearrange("p (t d) -> p t d", d=HD)
                    nc.sync.dma_start(
                        vt3, cv_d[bi, h].rearrange("(p t) d -> p t d", p=128)
                    )
                    nc.scalar.dma_start(
                        vo_d[bi, h, 0:KV, :].rearrange("(p t) d -> p t d", p=128),
                        vt3,
                    )
                    if "scores" in _SKIP:
                        continue
                    # broadcast q_bh across partitions via ones matmul
                    qb = ps_qb.tile([128, HD], FP, tag="qb")
                    nc.tensor.matmul(
                        qb[:], lhsT=ones_r[:],
                        rhs=row0["q"][0:1, bh * HD : (bh + 1) * HD],
                        start=True, stop=True,
                    )
                    tm = tmpp.tile([128, HT], FP, tag="tm")
                    t3 = tm[:].rearrange("p (t d) -> p t d", d=HD)
                    nc.vector.tensor_mul(
                        t3, kt3, qb[:, None, :].broadcast_to((128, NT, HD))
                    )
                    scores = small.tile([128, NT], FP, tag="scores")
                    nc.vector.tensor_reduce(scores[:], t3, axis=AX.X, op=OP.add)
                    pt = small.tile([128, NT], FP, tag="pt")
                    nc.scalar.activation(
                        pt[:], scores[:], ACT.Exp, scale=0.125,
                        accum_out=partials[:, bh : bh + 1],
                    )
                    if "attnv" in _SKIP:
                        continue
                    # per-head softmax denominator -> 1/denom broadcast to 64 rows
                    dn = ps_qb.tile([1, 1], FP, tag="qb")
                    nc.tensor.matmul(
                        dn[:], lhsT=partials[:, bh : bh + 1], rhs=ones_c[:],
                        start=True, stop=False,
                    )
                    nc.tensor.matmul(
                        dn[:], lhsT=pnew[0:1, bh : bh + 1], rhs=ones_r[0:1, 0:1],
                        start=False, stop=True,
                    )
                    inv1 = small.tile([1, 1], FP, tag="inv1")
                    nc.vector.reciprocal(inv1[:], dn[:])
                    inv64 = ps_proj.tile([HD, 1], FP, tag="proj")
                    nc.tensor.matmul(
                        inv64[:], lhsT=ones_r[0:1, 0:HD], rhs=inv1[:],
                        start=True, stop=True,
                    )
                    inv64_sb = small.tile([HD, 1], FP, tag="inv64_sb")
                    nc.vector.tensor_copy(inv64_sb[:], inv64[:])
                    # attn . V on the tensor engine: accumulating matmuls
                    yp = ps_y.tile([HD, 1], FP, tag="yp")
                    for t in range(NT):
                        nc.tensor.matmul(
                            yp[:],
                            lhsT=vt[:, t * HD : (t + 1) * HD],
                            rhs=pt[:, t : t + 1],
                            start=(t == 0), stop=False,
                        )
                    nc.tensor.matmul(
                        yp[:],
                        lhsT=row0["v"][0:1, bh * HD : (bh + 1) * HD],
                        rhs=pnew[0:1, bh : bh + 1],
                        start=False, stop=True,
                    )
                    # normalized y^T column (scale by 1/denom while copying out)
                    nc.vector.tensor_scalar_mul(
                        y_buf[:, bh : bh + 1], yp[:], inv64_sb[:]
                    )

            if "tail" not in _SKIP:
                # ---- output projection out = y @ Wo.T, contraction in 64-chunks
                # rhs columns come straight out of y_buf as strided views.
                y3 = y_buf[:].rearrange("p (b h) -> p h b", h=H)
                outT = consts.tile([128, 8 * BPC], FP, tag="outT")
                for m in range(8):
                    wblk = wpool.tile([128, D], FP, tag="wblk")
                    nc.sync.dma_start(wblk[:], wo_d[m * 128 : (m + 1) * 128, :])
                    op_ = ps_proj.tile([128, BPC], FP, tag="proj")
                    for h_ in range(H):
                        wt_ps = ps_misc.tile([128, 128], FP, tag="misc")
                        nc.tensor.transpose(
                            wt_ps[0:HD, 0:128],
                            wblk[:, h_ * HD : (h_ + 1) * HD],
                            ident[:],
                        )
                        wt_sb = small.tile([128, 128], FP, tag="wt_sb")
                        nc.vector.tensor_copy(wt_sb[0:HD, :], wt_ps[0:HD, 0:128])
                        nc.tensor.matmul(
                            op_[:],
                            lhsT=wt_sb[0:HD, :],
                            rhs=y3[:, h_, :],
                            start=(h_ == 0),
                            stop=(h_ == H - 1),
                        )
                    nc.vector.tensor_copy(outT[:, m * BPC : (m + 1) * BPC], op_[:])
                out_buf = consts.tile([BPC, D], FP, tag="rows")
                for m in range(8):
                    tp = ps_misc.tile([128, 512], FP, tag="misc")
                    nc.tensor.transpose(
                        tp[0:BPC, 0:128], outT[:, m * BPC : (m + 1) * BPC], ident[:]
                    )
                    nc.vector.tensor_copy(
                        out_buf[:, m * 128 : (m + 1) * 128], tp[0:BPC, 0:128]
                    )
                nc.sync.dma_start(out_d[:, 0, :], out_buf[:])

    nc.compile()
    _CACHED_NC = nc
    return nc


def _in_maps(x, cache_k, cache_v, Wq, Wk, Wv, Wo):
    x = np.asarray(x, dtype=np.float32)
    cache_k = np.asarray(cache_k, dtype=np.float32)
    cache_v = np.asarray(cache_v, dtype=np.float32)
    Wq = np.ascontiguousarray(np.asarray(Wq, dtype=np.float32))
    Wk = np.ascontiguousarray(np.asarray(Wk, dtype=np.float32))
    Wv = np.ascontiguousarray(np.asarray(Wv, dtype=np.float32))
    Wo = np.ascontiguousarray(np.asarray(Wo, dtype=np.float32))

    in_maps = []
    for c in range(N_CORES):
        sl = slice(c * BPC, (c + 1) * BPC)
        in_maps.append(
            {
                "x": np.ascontiguousarray(x[sl]),
                "cache_k": np.ascontiguousarray(cache_k[sl]),
                "cache_v": np.ascontiguousarray(cache_v[sl]),
                "Wq": Wq,
                "Wk": Wk,
                "Wv": Wv,
                "Wo": Wo,
            }
        )
    return in_maps


def _gather(results):
    out = np.concatenate([r["out"] for r in results], axis=0)
    k = np.concatenate([r["k"] for r in results], axis=0)
    v = np.concatenate([r["v"] for r in results], axis=0)
    return out, k, v


def kernel(x, cache_k, cache_v, Wq, Wk, Wv, Wo):
    from concourse.bass_utils import run_bass_kernel_spmd

    nc = _build()
    in_maps = _in_maps(x, cache_k, cache_v, Wq, Wk, Wv, Wo)
    res = run_bass_kernel_spmd(nc, in_maps, core_ids=list(range(N_CORES))).results
    return _gather(res)


# revision 15
# speedup vs baseline: 2.1458x; 1.0858x over previous
"""CacheMHA decode kernel for 8 TRN2 NeuronCores.

Sharding: data-parallel over batch (B=32 -> 4 per core). Each core computes the
full attention step for its 4 sequences (all 16 heads) plus the cache
concatenation, so no collectives are needed.

Per-core design (engine balance is the point):
  - DMA (the roofline, ~270 MB/core): cache K/V per (batch, head) streams
    HBM->SBUF as one 1 MiB DMA with 8 KiB contiguous runs (partition p holds
    seq rows p*32..p*32+31); the same SBUF tile is written back to the output
    cache, so each cache byte is read once and written once. Loads ride the
    SP HWDGE ring, stores the Activation ring.
  - DVE does only the q.K dot products (multiply + segmented reduce).
  - PE does everything contraction-shaped: weight transposes, projections,
    q broadcast, attn.V (PSUM-accumulated column matmuls), per-head softmax
    denominator sums, and the output projection.
  - ACT does only exp (fused denominator accumulation) and issues the store
    DMAs, so its function table never thrashes.
  - Softmax is normalized per-head as soon as that head's denominator is
    known, which removes any serial tail except the small output projection.
"""

import numpy as np

D = 1024
H = 16
HD = 64
B = 32
KV = 4096
N_CORES = 8
BPC = B // N_CORES  # batches per core

_CACHED_NC = None
_SKIP = frozenset()  # experiment toggles: "scores", "attnv", "tail"


def _build():
    global _CACHED_NC
    if _CACHED_NC is not None:
        return _CACHED_NC

    import concourse.bacc as bacc
    import concourse.mybir as mybir
    import concourse.tile as tile
    from concourse.masks import make_identity

    FP = mybir.dt.float32
    AX = mybir.AxisListType
    OP = mybir.AluOpType
    ACT = mybir.ActivationFunctionType

    nc = bacc.Bacc(
        "TRN2", target_bir_lowering=False, debug=False, num_devices=N_CORES
    )
    x_d = nc.declare_dram_parameter("x", [BPC, 1, D], FP, isOutput=False)
    ck_d = nc.declare_dram_parameter("cache_k", [BPC, H, KV, HD], FP, isOutput=False)
    cv_d = nc.declare_dram_parameter("cache_v", [BPC, H, KV, HD], FP, isOutput=False)
    wq_d = nc.declare_dram_parameter("Wq", [D, D], FP, isOutput=False)
    wk_d = nc.declare_dram_parameter("Wk", [D, D], FP, isOutput=False)
    wv_d = nc.declare_dram_parameter("Wv", [D, D], FP, isOutput=False)
    wo_d = nc.declare_dram_parameter("Wo", [D, D], FP, isOutput=False)
    out_d = nc.declare_dram_parameter("out", [BPC, 1, D], FP, isOutput=True)
    ko_d = nc.declare_dram_parameter("k", [BPC, H, KV + 1, HD], FP, isOutput=True)
    vo_d = nc.declare_dram_parameter("v", [BPC, H, KV + 1, HD], FP, isOutput=True)

    HT = KV // 128 * HD  # free size per head tile: 32*64 = 2048 f32
    NT = KV // 128  # 32 seq sub-rows per partition

    with tile.TileContext(nc) as tc:
        with (
            tc.tile_pool(name="consts", bufs=1) as consts,
            tc.tile_pool(name="small", bufs=3) as small,
            tc.tile_pool(name="kvpool", bufs=4) as kvp,
            tc.tile_pool(name="tmppool", bufs=2) as tmpp,
            tc.tile_pool(name="wpool", bufs=2) as wpool,
            tc.tile_pool(name="ps_qb", bufs=2, space="PSUM") as ps_qb,
            tc.tile_pool(name="ps_y", bufs=2, space="PSUM") as ps_y,
            tc.tile_pool(name="ps_proj", bufs=2, space="PSUM") as ps_proj,
            tc.tile_pool(name="ps_misc", bufs=2, space="PSUM") as ps_misc,
        ):
            ones_r = consts.tile([1, 128], FP, tag="ones_r")  # row of ones
            ones_c = consts.tile([128, 1], FP, tag="ones_c")  # column of ones
            ident = consts.tile([128, 128], FP, tag="ident")
            nc.gpsimd.memset(ones_r[:], 1.0)
            nc.gpsimd.memset(ones_c[:], 1.0)
            make_identity(nc, ident[:])

            # ---- x -> xT chunks [128, 4] x 8 (transpose on PE)
            x_sb = consts.tile([BPC, D], FP, tag="x_sb")
            nc.sync.dma_start(x_sb[:], x_d[:, 0, :])
            xT_all = consts.tile([128, 8 * BPC], FP, tag="xT_all")
            for k_ in range(8):
                tp = ps_misc.tile([128, BPC], FP, tag="misc")
                nc.tensor.transpose(
                    tp[:], x_sb[:, k_ * 128 : (k_ + 1) * 128], ident[0:BPC, 0:BPC]
                )
                nc.vector.tensor_copy(xT_all[:, k_ * BPC : (k_ + 1) * BPC], tp[:])

            # ---- projections q/k/v on PE: transpose W blocks, accumulate
            projTs = {}
            for wname, w_d in (("q", wq_d), ("k", wk_d), ("v", wv_d)):
                projT = consts.tile([128, 8 * BPC], FP, tag=f"{wname}T")
                projTs[wname] = projT
                for m in range(8):
                    wblk = wpool.tile([128, D], FP, tag="wblk")
                    nc.sync.dma_start(wblk[:], w_d[m * 128 : (m + 1) * 128, :])
                    pp = ps_proj.tile([128, BPC], FP, tag="proj")
                    for k_ in range(8):
                        wt_pool = ps_misc if k_ % 2 == 0 else ps_qb
                        wt_ps = wt_pool.tile(
                            [128, 128], FP, tag="misc" if k_ % 2 == 0 else "qb"
                        )
                        nc.tensor.transpose(
                            wt_ps[:], wblk[:, k_ * 128 : (k_ + 1) * 128], ident[:]
                        )
                        wt_sb = small.tile([128, 128], FP, tag="wt_sb")
                        nc.vector.tensor_copy(wt_sb[:], wt_ps[:])
                        nc.tensor.matmul(
                            pp[:],
                            lhsT=wt_sb[:],
                            rhs=xT_all[:, k_ * BPC : (k_ + 1) * BPC],
                            start=(k_ == 0),
                            stop=(k_ == 7),
                        )
                    nc.vector.tensor_copy(projT[:, m * BPC : (m + 1) * BPC], pp[:])

            # ---- projT -> row layout [BPC, D] -> row0 [1, 4096]
            row0 = {}
            for wname in ("q", "k", "v"):
                r = consts.tile([BPC, D], FP, tag=f"{wname}_rows")
                for m in range(8):
                    tp = ps_misc.tile([128, 512], FP, tag="misc")
                    nc.tensor.transpose(
                        tp[0:BPC, 0:128],
                        projTs[wname][:, m * BPC : (m + 1) * BPC],
                        ident[:],
                    )
                    nc.vector.tensor_copy(
                        r[:, m * 128 : (m + 1) * 128], tp[0:BPC, 0:128]
                    )
                r0 = consts.tile([1, BPC * D], FP, tag=f"{wname}_row0")
                row0[wname] = r0
                nc.sync.dma_start(r0[0:1, :], r[:])

            # ---- write new k/v rows into the output caches at seq position KV
            nc.scalar.dma_start(ko_d[:, :, KV, :], row0["k"][0:1, :])
            nc.scalar.dma_start(vo_d[:, :, KV, :], row0["v"][0:1, :])

            # ---- new-token scores: pnew[bh] = exp(0.125 * q_bh . knew_bh)
            sn_tmp = consts.tile([1, BPC * D], FP, tag="sn_tmp")
            nc.vector.tensor_mul(sn_tmp[:], row0["k"][:], row0["q"][:])
            snew = consts.tile([1, BPC * H], FP, tag="snew")
            nc.vector.tensor_reduce(
                snew[:], sn_tmp[:].rearrange("p (a d) -> p a d", d=HD),
                axis=AX.X, op=OP.add,
            )
            pnew = consts.tile([1, BPC * H], FP, tag="pnew")
            nc.scalar.activation(pnew[:], snew[:], ACT.Exp, scale=0.125)

            # ---- main loop over (batch, head): stream cache, attention
            partials = consts.tile([128, BPC * H], FP, tag="partials")
            y_buf = consts.tile([HD, BPC * H], FP, tag="y_buf")  # normalized y^T

            out_acc = consts.tile([128, 8 * BPC], FP, tag="out_acc")
            nc.gpsimd.memset(out_acc[:], 0.0)
            y3 = y_buf[:].rearrange("p (b h) -> p h b", h=H)
            for h in range(H):
                for bi in range(BPC):
                    bh = bi * H + h
                    kt = kvp.tile([128, HT], FP, tag="kt")
                    kt3 = kt[:].rearrange("p (t d) -> p t d", d=HD)
                    nc.sync.dma_start(
                        kt3, ck_d[bi, h].rearrange("(p t) d -> p t d", p=128)
                    )
                    nc.scalar.dma_start(
                        ko_d[bi, h, 0:KV, :].rearrange("(p t) d -> p t d", p=128),
                        kt3,
                    )
                    vt = kvp.tile([128, HT], FP, tag="vt")
                    vt3 = vt[:].rearrange("p (t d) -> p t d", d=HD)
                    nc.sync.dma_start(
                        vt3, cv_d[bi, h].rearrange("(p t) d -> p t d", p=128)
                    )
                    nc.scalar.dma_start(
                        vo_d[bi, h, 0:KV, :].rearrange("(p t) d -> p t d", p=128),
                        vt3,
                    )
                    if "scores" in _SKIP:
                        continue
                    # broadcast q_bh across partitions via ones matmul
                    qb = ps_qb.tile([128, HD], FP, tag="qb")
                    nc.tensor.matmul(
                        qb[:], lhsT=ones_r[:],
                        rhs=row0["q"][0:1, bh * HD : (bh + 1) * HD],
                        start=True, stop=True,
                    )
                    tm = tmpp.tile([128, HT], FP, tag="tm")
                    t3 = tm[:].rearrange("p (t d) -> p t d", d=HD)
                    nc.vector.tensor_mul(
                        t3, kt3, qb[:, None, :].broadcast_to((128, NT, HD))
                    )
                    scores = small.tile([128, NT], FP, tag="scores")
                    nc.vector.tensor_reduce(scores[:], t3, axis=AX.X, op=OP.add)
                    pt = small.tile([128, NT], FP, tag="pt")
                    nc.scalar.activation(
                        pt[:], scores[:], ACT.Exp, scale=0.125,
                        accum_out=partials[:, bh : bh + 1],
                    )
                    if "attnv" in _SKIP:
                        continue
                    # per-head softmax denominator -> 1/denom broadcast to 64 rows
                    dn = ps_misc.tile([1, 1], FP, tag="misc")
                    nc.tensor.matmul(
                        dn[:], lhsT=partials[:, bh : bh + 1], rhs=ones_c[:],
                        start=True, stop=False,
                    )
                    nc.tensor.matmul(
                        dn[:], lhsT=pnew[0:1, bh : bh + 1], rhs=ones_r[0:1, 0:1],
                        start=False, stop=True,
                    )
                    inv1 = small.tile([1, 1], FP, tag="inv1")
                    nc.vector.reciprocal(inv1[:], dn[:])
                    inv64 = ps_proj.tile([HD, 1], FP, tag="proj")
                    nc.tensor.matmul(
                        inv64[:], lhsT=ones_r[0:1, 0:HD], rhs=inv1[:],
                        start=True, stop=True,
                    )
                    inv64_sb = small.tile([HD, 1], FP, tag="inv64_sb")
                    nc.vector.tensor_copy(inv64_sb[:], inv64[:])
                    # attn . V on the tensor engine: accumulating matmuls
                    yp = ps_y.tile([HD, 1], FP, tag="yp")
                    for t in range(NT):
                        nc.tensor.matmul(
                            yp[:],
                            lhsT=vt[:, t * HD : (t + 1) * HD],
                            rhs=pt[:, t : t + 1],
                            start=(t == 0), stop=False,
                        )
                    nc.tensor.matmul(
                        yp[:],
                        lhsT=row0["v"][0:1, bh * HD : (bh + 1) * HD],
                        rhs=pnew[0:1, bh : bh + 1],
                        start=False, stop=True,
                    )
                    # normalized y^T column (scale by 1/denom while copying out)
                    nc.vector.tensor_scalar_mul(
                        y_buf[:, bh : bh + 1], yp[:], inv64_sb[:]
                    )

                if "tail" in _SKIP or "attnv" in _SKIP or "scores" in _SKIP:
                    continue
                # out += y[:, h] @ WoT[h-chunk] as soon as head h is complete.
                # Wo streams in as [128, 256] quarter-blocks, one group per 4 heads.
                if h % 4 == 0:
                    wo_grp = []
                    for m in range(8):
                        woq = wpool.tile([128, 4 * HD], FP, tag="woq", bufs=16)
                        nc.sync.dma_start(
                            woq[:],
                            wo_d[m * 128 : (m + 1) * 128, h * HD : (h + 4) * HD],
                        )
                        wo_grp.append(woq)
                for m in range(8):
                    wt_ps = ps_misc.tile([128, 128], FP, tag="misc")
                    nc.tensor.transpose(
                        wt_ps[0:HD, 0:128],
                        wo_grp[m][:, (h % 4) * HD : (h % 4 + 1) * HD],
                        ident[:],
                    )
                    wt_sb = small.tile([128, 128], FP, tag="wt_sb")
                    nc.vector.tensor_copy(wt_sb[0:HD, :], wt_ps[0:HD, 0:128])
                    op_ = ps_proj.tile([128, BPC], FP, tag="proj")
                    nc.tensor.matmul(
                        op_[:], lhsT=wt_sb[0:HD, :], rhs=y3[:, h, :],
                        start=True, stop=True,
                    )
                    nc.vector.tensor_add(
                        out_acc[:, m * BPC : (m + 1) * BPC],
                        out_acc[:, m * BPC : (m + 1) * BPC],
                        op_[:],
                    )

            if "tail" not in _SKIP and "attnv" not in _SKIP and "scores" not in _SKIP:
                out_buf = consts.tile([BPC, D], FP, tag="q_rows")
                for m in range(8):
                    tp = ps_misc.tile([128, 512], FP, tag="misc")
                    nc.tensor.transpose(
                        tp[0:BPC, 0:128], out_acc[:, m * BPC : (m + 1) * BPC], ident[:]
                    )
                    nc.vector.tensor_copy(
                        out_buf[:, m * 128 : (m + 1) * 128], tp[0:BPC, 0:128]
                    )
                nc.sync.dma_start(out_d[:, 0, :], out_buf[:])

    nc.compile()
    _CACHED_NC = nc
    return nc


def _in_maps(x, cache_k, cache_v, Wq, Wk, Wv, Wo):
    x = np.asarray(x, dtype=np.float32)
    cache_k = np.asarray(cache_k, dtype=np.float32)
    cache_v = np.asarray(cache_v, dtype=np.float32)
    Wq = np.ascontiguousarray(np.asarray(Wq, dtype=np.float32))
    Wk = np.ascontiguousarray(np.asarray(Wk, dtype=np.float32))
    Wv = np.ascontiguousarray(np.asarray(Wv, dtype=np.float32))
    Wo = np.ascontiguousarray(np.asarray(Wo, dtype=np.float32))

    in_maps = []
    for c in range(N_CORES):
        sl = slice(c * BPC, (c + 1) * BPC)
        in_maps.append(
            {
                "x": np.ascontiguousarray(x[sl]),
                "cache_k": np.ascontiguousarray(cache_k[sl]),
                "cache_v": np.ascontiguousarray(cache_v[sl]),
                "Wq": Wq,
                "Wk": Wk,
                "Wv": Wv,
                "Wo": Wo,
            }
        )
    return in_maps


def _gather(results):
    out = np.concatenate([r["out"] for r in results], axis=0)
    k = np.concatenate([r["k"] for r in results], axis=0)
    v = np.concatenate([r["v"] for r in results], axis=0)
    return out, k, v


def kernel(x, cache_k, cache_v, Wq, Wk, Wv, Wo):
    from concourse.bass_utils import run_bass_kernel_spmd

    nc = _build()
    in_maps = _in_maps(x, cache_k, cache_v, Wq, Wk, Wv, Wo)
    res = run_bass_kernel_spmd(nc, in_maps, core_ids=list(range(N_CORES))).results
    return _gather(res)
